# revision 5
# baseline (speedup 1.0000x reference)
# DropConnect LSTM cell kernel for Trainium2 (Bass/Tile), data-parallel over
# batch across 8 NeuronCores.
#
# Math (per reference):
#   x_d = x * (dp_u >= 0.1) / 0.9
#   h_d = h * (rec_dp_u >= 0.1) / 0.9
#   w   = kernel * (k_dp_u >= 0.05) / 0.95
#   rw  = recurrent_kernel * (rk_dp_u >= 0.05) / 0.95
#   z   = x_d @ w + h_d @ rw + bias          (split into gates i,f,c~,o)
#   c'  = sig(zf)*c + sig(zi)*tanh(zc)
#   h'  = sig(zo)*tanh(c')
#
# Kernel strategy (per core, B_c = 1024 batch rows):
#  - Both 1/(1-rate) scales are identical for the x and h paths, so the
#    combined scale S = 1/(0.9*0.95) is applied once inside the gate
#    activations (out = f(S*psum)), and bias is pre-divided by S and
#    injected into PSUM with a K=1 matmul so psum = act_m@w_m + bias/S.
#  - Masks applied with one fused DVE scalar_tensor_tensor:
#    out = (u >= rate) * v.
#  - Activations are transposed on-chip with PE transposes into an
#    actT[K=2048, B_c] buffer; weights stream in natural layout.
#  - Matmuls run in float32r (full fp32 inputs, 1 cycle/row at N=512).
#  - Gate-ordered chunk loop (c~ -> i -> f -> o) so i*tanh(zc) and c' can be
#    accumulated in a single resident buffer.

from contextlib import ExitStack

import numpy as np

import concourse.bass as bass
import concourse.mybir as mybir
import concourse.tile as tile
from concourse import bacc
from concourse.bass_utils import run_bass_kernel_spmd
from concourse.masks import make_identity

N_CORES = 8
B, D, U = 8192, 1024, 1024
BC = B // N_CORES  # per-core batch rows
P = 128
NG4 = 4 * U  # 4096 gate columns
KT = (D + U) // P  # 16 contraction tiles
NW = 512  # matmul free-dim chunk

DROPOUT = 0.1
KERNEL_DROPOUT = 0.05
S = 1.0 / ((1.0 - DROPOUT) * (1.0 - KERNEL_DROPOUT))

f32 = mybir.dt.float32
f32r = mybir.dt.float32r
AF = mybir.ActivationFunctionType
OP = mybir.AluOpType


def build_nc(bc: int = BC):
    """Build and compile the per-core Bass program for per-core batch bc."""
    btl = bc // P
    nc = bacc.Bacc("TRN2", target_bir_lowering=False, debug=False)

    x = nc.dram_tensor("x", [bc, D], f32, kind="ExternalInput").ap()
    h = nc.dram_tensor("h", [bc, U], f32, kind="ExternalInput").ap()
    c_in = nc.dram_tensor("c", [bc, U], f32, kind="ExternalInput").ap()
    dp = nc.dram_tensor("dp_u", [bc, D], f32, kind="ExternalInput").ap()
    rdp = nc.dram_tensor("rec_dp_u", [bc, U], f32, kind="ExternalInput").ap()
    kw = nc.dram_tensor("kern", [D, NG4], f32, kind="ExternalInput").ap()
    rkw = nc.dram_tensor("rkern", [U, NG4], f32, kind="ExternalInput").ap()
    kdp = nc.dram_tensor("k_dp_u", [D, NG4], f32, kind="ExternalInput").ap()
    rkdp = nc.dram_tensor("rk_dp_u", [U, NG4], f32, kind="ExternalInput").ap()
    bias = nc.dram_tensor("bias", [NG4], f32, kind="ExternalInput").ap()
    h_new = nc.dram_tensor("h_new", [bc, U], f32, kind="ExternalOutput").ap()
    c_new = nc.dram_tensor("c_new", [bc, U], f32, kind="ExternalOutput").ap()

    with tile.TileContext(nc) as tc, ExitStack() as ctx:
        const = ctx.enter_context(tc.tile_pool(name="const", bufs=1))
        astage = ctx.enter_context(tc.tile_pool(name="astage", bufs=4))
        atrans = ctx.enter_context(tc.tile_pool(name="atrans", bufs=1))
        wstage = ctx.enter_context(tc.tile_pool(name="wstage", bufs=8))
        wmpool = ctx.enter_context(tc.tile_pool(name="wm", bufs=4))
        gstage = ctx.enter_context(tc.tile_pool(name="gstage", bufs=8))
        tpool = ctx.enter_context(tc.tile_pool(name="tpool", bufs=1))
        psum = ctx.enter_context(tc.tile_pool(name="psum", bufs=8, space="PSUM"))

        ident = const.tile([P, P], f32)
        make_identity(nc, ident)
        # f32r-typed operands: fp32r matmuls require producers that round to
        # fp32r, so every matmul operand tile is written with dtype float32r.
        ones_raw = const.tile([1, P], f32)
        nc.vector.memset(ones_raw, 1.0)
        ones1 = const.tile([1, P], f32r)
        nc.vector.tensor_copy(ones1, ones_raw)
        bias_raw = const.tile([1, NG4], f32)
        nc.sync.dma_start(out=bias_raw, in_=bias.unsqueeze(0))
        bias_s = const.tile([1, NG4], f32r)
        nc.vector.tensor_scalar_mul(bias_s, bias_raw, 1.0 / S)

        # actT[:, kk, j] = masked activation, contraction row kk*128.., batch col j
        actT = atrans.tile([P, KT, bc], f32r)

        # ---- Phase 1: mask activations and transpose them into actT ----
        for bt in range(btl):
            rows = slice(bt * P, (bt + 1) * P)
            for src, usrc, kbase in ((x, dp, 0), (h, rdp, KT // 2)):
                vt = astage.tile([P, D], f32, tag="araw")
                ut = astage.tile([P, D], f32, tag="araw")
                nc.sync.dma_start(out=vt, in_=src[rows, :])
                nc.sync.dma_start(out=ut, in_=usrc[rows, :])
                vm = astage.tile([P, D], f32, tag="amask")
                nc.vector.scalar_tensor_tensor(
                    vm, ut, DROPOUT, vt, op0=OP.is_ge, op1=OP.mult
                )
                for grp in range(2):
                    pt = psum.tile([P, 4, P], f32, tag="ps")
                    for q in range(4):
                        j = grp * 4 + q
                        nc.tensor.transpose(
                            pt[:, q, :], vm[:, j * P : (j + 1) * P], ident
                        )
                    dst = actT[:, kbase + grp * 4 : kbase + grp * 4 + 4, rows]
                    nc.scalar.copy(dst, pt)

        # ---- Phase 2: matmul chunks + gate math, in gate order c~, i, f, o ----
        # T_all[:, b, :] holds tanh(zc), then i*tanh(zc), then c', per b-tile.
        T_all = tpool.tile([P, btl, U], f32)

        gate_order = [(2, 0), (2, 1), (0, 0), (0, 1), (1, 0), (1, 1), (3, 0), (3, 1)]
        for g, hf in gate_order:
            col0 = g * U + hf * NW
            ucols = slice(hf * NW, (hf + 1) * NW)
            zp = []
            for b in range(btl):
                zb = psum.tile([P, NW], f32, tag="ps")
                zp.append(zb)
                nc.tensor.matmul(
                    zb,
                    lhsT=ones1,
                    rhs=bias_s[:, col0 : col0 + NW],
                    start=True,
                    stop=False,
                )
            for kk in range(KT):
                wsrc = kw if kk < KT // 2 else rkw
                usrc = kdp if kk < KT // 2 else rkdp
                r0 = (kk % (KT // 2)) * P
                wt = wstage.tile([P, NW], f32, tag="wraw")
                uw = wstage.tile([P, NW], f32, tag="wraw")
                nc.sync.dma_start(out=wt, in_=wsrc[r0 : r0 + P, col0 : col0 + NW])
                nc.sync.dma_start(out=uw, in_=usrc[r0 : r0 + P, col0 : col0 + NW])
                wm = wmpool.tile([P, NW], f32r)
                nc.vector.scalar_tensor_tensor(
                    wm, uw, KERNEL_DROPOUT, wt, op0=OP.is_ge, op1=OP.mult
                )
                wmr = wm
                for b in range(btl):
                    nc.tensor.matmul(
                        zp[b],
                        lhsT=actT[:, kk, b * P : (b + 1) * P],
                        rhs=wmr,
                        start=False,
                        stop=(kk == KT - 1),
                    )
            for b in range(btl):
                rows = slice(b * P, (b + 1) * P)
                tsl = T_all[:, b, ucols]
                if g == 2:  # candidate: T = tanh(zc)
                    nc.scalar.activation(tsl, zp[b], AF.Tanh, scale=S)
                elif g == 0:  # input gate: T = sig(zi) * T
                    sg = gstage.tile([P, NW], f32, tag="g")
                    nc.scalar.activation(sg, zp[b], AF.Sigmoid, scale=S)
                    nc.vector.tensor_tensor(tsl, sg, tsl, OP.mult)
                elif g == 1:  # forget gate: c' = sig(zf)*c + T, store c'
                    sg = gstage.tile([P, NW], f32, tag="g")
                    nc.scalar.activation(sg, zp[b], AF.Sigmoid, scale=S)
                    ct = gstage.tile([P, NW], f32, tag="g")
                    nc.sync.dma_start(out=ct, in_=c_in[rows, ucols])
                    nc.vector.tensor_tensor(sg, sg, ct, OP.mult)
                    nc.vector.tensor_tensor(tsl, tsl, sg, OP.add)
                    nc.sync.dma_start(out=c_new[rows, ucols], in_=tsl)
                else:  # output gate: h' = sig(zo) * tanh(c')
                    sg = gstage.tile([P, NW], f32, tag="g")
                    nc.scalar.activation(sg, zp[b], AF.Sigmoid, scale=S)
                    tct = gstage.tile([P, NW], f32, tag="g")
                    nc.scalar.activation(tct, tsl, AF.Tanh)
                    nc.vector.tensor_tensor(tct, sg, tct, OP.mult)
                    nc.sync.dma_start(out=h_new[rows, ucols], in_=tct)

    nc.compile()
    return nc


_NC_CACHE: dict[int, object] = {}


def get_nc(bc: int = BC):
    if bc not in _NC_CACHE:
        _NC_CACHE[bc] = build_nc(bc)
    return _NC_CACHE[bc]


def make_in_maps(x, h, c, kernel, recurrent_kernel, bias, dp_u, rec_dp_u, k_dp_u, rk_dp_u):
    def f(a):
        return np.ascontiguousarray(np.asarray(a, dtype=np.float32))

    kernel = f(kernel)
    recurrent_kernel = f(recurrent_kernel)
    bias = f(bias)
    k_dp_u = f(k_dp_u)
    rk_dp_u = f(rk_dp_u)
    x, h, c, dp_u, rec_dp_u = f(x), f(h), f(c), f(dp_u), f(rec_dp_u)

    in_maps = []
    for ci in range(N_CORES):
        sl = slice(ci * BC, (ci + 1) * BC)
        in_maps.append(
            {
                "x": np.ascontiguousarray(x[sl]),
                "h": np.ascontiguousarray(h[sl]),
                "c": np.ascontiguousarray(c[sl]),
                "dp_u": np.ascontiguousarray(dp_u[sl]),
                "rec_dp_u": np.ascontiguousarray(rec_dp_u[sl]),
                "kern": kernel,
                "rkern": recurrent_kernel,
                "k_dp_u": k_dp_u,
                "rk_dp_u": rk_dp_u,
                "bias": bias,
            }
        )
    return in_maps


def kernel(x, h, c, kernel, recurrent_kernel, bias, dp_u, rec_dp_u, k_dp_u, rk_dp_u):
    nc = get_nc()
    in_maps = make_in_maps(
        x, h, c, kernel, recurrent_kernel, bias, dp_u, rec_dp_u, k_dp_u, rk_dp_u
    )
    res = run_bass_kernel_spmd(nc, in_maps, core_ids=list(range(N_CORES)))
    h_new = np.concatenate([res.results[ci]["h_new"] for ci in range(N_CORES)], axis=0)
    c_new = np.concatenate([res.results[ci]["c_new"] for ci in range(N_CORES)], axis=0)
    return h_new, c_new


# revision 8
# speedup vs baseline: 14.5797x; 14.5797x over previous
# DropConnect LSTM cell kernel for Trainium2 (Bass/Tile), data-parallel over
# batch across 8 NeuronCores.
#
# Math (per reference):
#   x_d = x * (dp_u >= 0.1) / 0.9
#   h_d = h * (rec_dp_u >= 0.1) / 0.9
#   w   = kernel * (k_dp_u >= 0.05) / 0.95
#   rw  = recurrent_kernel * (rk_dp_u >= 0.05) / 0.95
#   z   = x_d @ w + h_d @ rw + bias          (split into gates i,f,c~,o)
#   c'  = sig(zf)*c + sig(zi)*tanh(zc)
#   h'  = sig(zo)*tanh(c')
#
# Kernel strategy (per core, B_c = 1024 batch rows):
#  - Both 1/(1-rate) scales are identical for the x and h paths, so the
#    combined scale S = 1/(0.9*0.95) is applied once inside the gate
#    activations (out = f(S*psum)), and bias is pre-divided by S and
#    injected into PSUM with a K=1 matmul so psum = act_m@w_m + bias/S.
#  - Masks applied with one fused DVE scalar_tensor_tensor:
#    out = (u >= rate) * v.
#  - Activations are transposed on-chip with PE transposes into an
#    actT[K=2048, B_c] buffer; weights stream in natural layout.
#  - Matmuls run in float32r (full fp32 inputs, 1 cycle/row at N=512).
#  - Gate-ordered chunk loop (c~ -> i -> f -> o) so i*tanh(zc) and c' can be
#    accumulated in a single resident buffer.

from contextlib import ExitStack

import numpy as np

import concourse.bass as bass
import concourse.mybir as mybir
import concourse.tile as tile
from concourse import bacc
from concourse.bass_utils import run_bass_kernel_spmd
from concourse.masks import make_identity

N_CORES = 8
B, D, U = 8192, 1024, 1024
BC = B // N_CORES  # per-core batch rows
P = 128
NG4 = 4 * U  # 4096 gate columns
KT = (D + U) // P  # 16 contraction tiles
NW = 512  # matmul free-dim chunk

DROPOUT = 0.1
KERNEL_DROPOUT = 0.05
S = 1.0 / ((1.0 - DROPOUT) * (1.0 - KERNEL_DROPOUT))

f32 = mybir.dt.float32
f32r = mybir.dt.float32r
AF = mybir.ActivationFunctionType
OP = mybir.AluOpType


def build_nc(bc: int = BC, repeat: int = 1):
    """Build and compile the per-core Bass program for per-core batch bc.

    repeat > 1 re-emits the whole computation N times in one NEFF (same
    inputs/outputs) — used only for device-time measurement in test.py.
    """
    btl = bc // P
    nc = bacc.Bacc("TRN2", target_bir_lowering=False, debug=False)

    x = nc.dram_tensor("x", [bc, D], f32, kind="ExternalInput").ap()
    h = nc.dram_tensor("h", [bc, U], f32, kind="ExternalInput").ap()
    c_in = nc.dram_tensor("c", [bc, U], f32, kind="ExternalInput").ap()
    dp = nc.dram_tensor("dp_u", [bc, D], f32, kind="ExternalInput").ap()
    rdp = nc.dram_tensor("rec_dp_u", [bc, U], f32, kind="ExternalInput").ap()
    kw = nc.dram_tensor("kern", [D, NG4], f32, kind="ExternalInput").ap()
    rkw = nc.dram_tensor("rkern", [U, NG4], f32, kind="ExternalInput").ap()
    kdp = nc.dram_tensor("k_dp_u", [D, NG4], f32, kind="ExternalInput").ap()
    rkdp = nc.dram_tensor("rk_dp_u", [U, NG4], f32, kind="ExternalInput").ap()
    bias = nc.dram_tensor("bias", [NG4], f32, kind="ExternalInput").ap()
    h_new = nc.dram_tensor("h_new", [bc, U], f32, kind="ExternalOutput").ap()
    c_new = nc.dram_tensor("c_new", [bc, U], f32, kind="ExternalOutput").ap()

    with tile.TileContext(nc) as tc, ExitStack() as ctx:
        const = ctx.enter_context(tc.tile_pool(name="const", bufs=1))
        astage = ctx.enter_context(tc.tile_pool(name="astage", bufs=4))
        atrans = ctx.enter_context(tc.tile_pool(name="atrans", bufs=1))
        wstage = ctx.enter_context(tc.tile_pool(name="wstage", bufs=8))
        wmpool = ctx.enter_context(tc.tile_pool(name="wm", bufs=4))
        gstage = ctx.enter_context(tc.tile_pool(name="gstage", bufs=8))
        tpool = ctx.enter_context(tc.tile_pool(name="tpool", bufs=1))
        psum = ctx.enter_context(tc.tile_pool(name="psum", bufs=8, space="PSUM"))

        ident = const.tile([P, P], f32)
        make_identity(nc, ident)
        # f32r-typed operands: fp32r matmuls require producers that round to
        # fp32r, so every matmul operand tile is written with dtype float32r.
        ones_raw = const.tile([1, P], f32)
        nc.vector.memset(ones_raw, 1.0)
        ones1 = const.tile([1, P], f32r)
        nc.vector.tensor_copy(ones1, ones_raw)
        bias_raw = const.tile([1, NG4], f32)
        nc.sync.dma_start(out=bias_raw, in_=bias.unsqueeze(0))
        bias_s = const.tile([1, NG4], f32r)
        nc.vector.tensor_scalar_mul(bias_s, bias_raw, 1.0 / S)

        for _rep in range(repeat):
            emit_body(
                nc, tc, btl, bc,
                x, h, c_in, dp, rdp, kw, rkw, kdp, rkdp, h_new, c_new,
                astage, atrans, wstage, wmpool, gstage, tpool, psum,
                ident, ones1, bias_s,
            )

    nc.compile()
    return nc


def emit_body(
    nc, tc, btl, bc,
    x, h, c_in, dp, rdp, kw, rkw, kdp, rkdp, h_new, c_new,
    astage, atrans, wstage, wmpool, gstage, tpool, psum,
    ident, ones1, bias_s,
):
        # actT[:, kk, j] = masked activation, contraction row kk*128.., batch col j
        actT = atrans.tile([P, KT, bc], f32r, tag="actT")

        # ---- Phase 1: mask activations and transpose them into actT ----
        for bt in range(btl):
            rows = slice(bt * P, (bt + 1) * P)
            for src, usrc, kbase in ((x, dp, 0), (h, rdp, KT // 2)):
                vt = astage.tile([P, D], f32, tag="araw")
                ut = astage.tile([P, D], f32, tag="araw")
                nc.sync.dma_start(out=vt, in_=src[rows, :])
                nc.sync.dma_start(out=ut, in_=usrc[rows, :])
                vm = astage.tile([P, D], f32, tag="amask")
                nc.vector.scalar_tensor_tensor(
                    vm, ut, DROPOUT, vt, op0=OP.is_ge, op1=OP.mult
                )
                for grp in range(2):
                    pt = psum.tile([P, 4, P], f32, tag="ps")
                    for q in range(4):
                        j = grp * 4 + q
                        nc.tensor.transpose(
                            pt[:, q, :], vm[:, j * P : (j + 1) * P], ident
                        )
                    dst = actT[:, kbase + grp * 4 : kbase + grp * 4 + 4, rows]
                    nc.scalar.copy(dst, pt)

        # ---- Phase 2: matmul chunks + gate math, in gate order c~, i, f, o ----
        # T_all[:, b, :] holds tanh(zc), then i*tanh(zc), then c', per b-tile.
        T_all = tpool.tile([P, btl, U], f32)

        gate_order = [(2, 0), (2, 1), (0, 0), (0, 1), (1, 0), (1, 1), (3, 0), (3, 1)]
        for g, hf in gate_order:
            col0 = g * U + hf * NW
            ucols = slice(hf * NW, (hf + 1) * NW)
            zp = []
            for b in range(btl):
                zb = psum.tile([P, NW], f32, tag="ps")
                zp.append(zb)
                nc.tensor.matmul(
                    zb,
                    lhsT=ones1,
                    rhs=bias_s[:, col0 : col0 + NW],
                    start=True,
                    stop=False,
                )
            for kk in range(KT):
                wsrc = kw if kk < KT // 2 else rkw
                usrc = kdp if kk < KT // 2 else rkdp
                r0 = (kk % (KT // 2)) * P
                wt = wstage.tile([P, NW], f32, tag="wraw")
                uw = wstage.tile([P, NW], f32, tag="wraw")
                nc.sync.dma_start(out=wt, in_=wsrc[r0 : r0 + P, col0 : col0 + NW])
                nc.sync.dma_start(out=uw, in_=usrc[r0 : r0 + P, col0 : col0 + NW])
                wm = wmpool.tile([P, NW], f32r)
                nc.vector.scalar_tensor_tensor(
                    wm, uw, KERNEL_DROPOUT, wt, op0=OP.is_ge, op1=OP.mult
                )
                wmr = wm
                for b in range(btl):
                    nc.tensor.matmul(
                        zp[b],
                        lhsT=actT[:, kk, b * P : (b + 1) * P],
                        rhs=wmr,
                        start=False,
                        stop=(kk == KT - 1),
                    )
            for b in range(btl):
                rows = slice(b * P, (b + 1) * P)
                tsl = T_all[:, b, ucols]
                if g == 2:  # candidate: T = tanh(zc)
                    nc.scalar.activation(tsl, zp[b], AF.Tanh, scale=S)
                elif g == 0:  # input gate: T = sig(zi) * T
                    sg = gstage.tile([P, NW], f32, tag="g")
                    nc.scalar.activation(sg, zp[b], AF.Sigmoid, scale=S)
                    nc.vector.tensor_tensor(tsl, sg, tsl, OP.mult)
                elif g == 1:  # forget gate: c' = sig(zf)*c + T, store c'
                    sg = gstage.tile([P, NW], f32, tag="g")
                    nc.scalar.activation(sg, zp[b], AF.Sigmoid, scale=S)
                    ct = gstage.tile([P, NW], f32, tag="g")
                    nc.sync.dma_start(out=ct, in_=c_in[rows, ucols])
                    nc.vector.tensor_tensor(sg, sg, ct, OP.mult)
                    nc.vector.tensor_tensor(tsl, tsl, sg, OP.add)
                    nc.sync.dma_start(out=c_new[rows, ucols], in_=tsl)
                else:  # output gate: h' = sig(zo) * tanh(c')
                    sg = gstage.tile([P, NW], f32, tag="g")
                    nc.scalar.activation(sg, zp[b], AF.Sigmoid, scale=S)
                    tct = gstage.tile([P, NW], f32, tag="g")
                    nc.scalar.activation(tct, tsl, AF.Tanh)
                    nc.vector.tensor_tensor(tct, sg, tct, OP.mult)
                    nc.sync.dma_start(out=h_new[rows, ucols], in_=tct)


_NC_CACHE: dict[tuple, object] = {}


def get_nc(bc: int = BC, repeat: int = 1):
    key = (bc, repeat)
    if key not in _NC_CACHE:
        _NC_CACHE[key] = build_nc(bc, repeat)
    return _NC_CACHE[key]


def make_in_maps(x, h, c, kernel, recurrent_kernel, bias, dp_u, rec_dp_u, k_dp_u, rk_dp_u):
    def f(a):
        return np.ascontiguousarray(np.asarray(a, dtype=np.float32))

    kernel = f(kernel)
    recurrent_kernel = f(recurrent_kernel)
    bias = f(bias)
    k_dp_u = f(k_dp_u)
    rk_dp_u = f(rk_dp_u)
    x, h, c, dp_u, rec_dp_u = f(x), f(h), f(c), f(dp_u), f(rec_dp_u)

    in_maps = []
    for ci in range(N_CORES):
        sl = slice(ci * BC, (ci + 1) * BC)
        in_maps.append(
            {
                "x": np.ascontiguousarray(x[sl]),
                "h": np.ascontiguousarray(h[sl]),
                "c": np.ascontiguousarray(c[sl]),
                "dp_u": np.ascontiguousarray(dp_u[sl]),
                "rec_dp_u": np.ascontiguousarray(rec_dp_u[sl]),
                "kern": kernel,
                "rkern": recurrent_kernel,
                "k_dp_u": k_dp_u,
                "rk_dp_u": rk_dp_u,
                "bias": bias,
            }
        )
    return in_maps


def kernel(x, h, c, kernel, recurrent_kernel, bias, dp_u, rec_dp_u, k_dp_u, rk_dp_u):
    nc = get_nc()
    in_maps = make_in_maps(
        x, h, c, kernel, recurrent_kernel, bias, dp_u, rec_dp_u, k_dp_u, rk_dp_u
    )
    res = run_bass_kernel_spmd(nc, in_maps, core_ids=list(range(N_CORES)))
    h_new = np.concatenate([res.results[ci]["h_new"] for ci in range(N_CORES)], axis=0)
    c_new = np.concatenate([res.results[ci]["c_new"] for ci in range(N_CORES)], axis=0)
    return h_new, c_new


# revision 9
# speedup vs baseline: 121.4317x; 8.3288x over previous
# DropConnect LSTM cell kernel for Trainium2 (Bass/Tile), data-parallel over
# batch across 8 NeuronCores.
#
# Math (per reference):
#   x_d = x * (dp_u >= 0.1) / 0.9
#   h_d = h * (rec_dp_u >= 0.1) / 0.9
#   w   = kernel * (k_dp_u >= 0.05) / 0.95
#   rw  = recurrent_kernel * (rk_dp_u >= 0.05) / 0.95
#   z   = x_d @ w + h_d @ rw + bias          (split into gates i,f,c~,o)
#   c'  = sig(zf)*c + sig(zi)*tanh(zc)
#   h'  = sig(zo)*tanh(c')
#
# Kernel strategy (per core, B_c = 1024 batch rows):
#  - Both 1/(1-rate) scales are identical for the x and h paths, so the
#    combined scale S = 1/(0.9*0.95) is applied once inside the gate
#    activations (out = f(S*psum)), and bias is pre-divided by S and
#    injected into PSUM with a K=1 matmul so psum = act_m@w_m + bias/S.
#  - Masks applied with one fused DVE scalar_tensor_tensor:
#    out = (u >= rate) * v.
#  - Activations are transposed on-chip with PE transposes into an
#    actT[K=2048, B_c] buffer; weights stream in natural layout.
#  - Matmuls run in float32r (full fp32 inputs, 1 cycle/row at N=512).
#  - Gate-ordered chunk loop (c~ -> i -> f -> o) so i*tanh(zc) and c' can be
#    accumulated in a single resident buffer.

from contextlib import ExitStack

import numpy as np

import concourse.bass as bass
import concourse.mybir as mybir
import concourse.tile as tile
from concourse import bacc
from concourse.bass_utils import run_bass_kernel_spmd
from concourse.masks import make_identity

N_CORES = 8
B, D, U = 8192, 1024, 1024
BC = B // N_CORES  # per-core batch rows
P = 128
NG4 = 4 * U  # 4096 gate columns
KT = (D + U) // P  # 16 contraction tiles
NW = 512  # matmul free-dim chunk

DROPOUT = 0.1
KERNEL_DROPOUT = 0.05
S = 1.0 / ((1.0 - DROPOUT) * (1.0 - KERNEL_DROPOUT))

f32 = mybir.dt.float32
f32r = mybir.dt.float32r
AF = mybir.ActivationFunctionType
OP = mybir.AluOpType


def build_nc(bc: int = BC, repeat: int = 1):
    """Build and compile the per-core Bass program for per-core batch bc.

    repeat > 1 re-emits the whole computation N times in one NEFF (same
    inputs/outputs) — used only for device-time measurement in test.py.
    """
    btl = bc // P
    nc = bacc.Bacc("TRN2", target_bir_lowering=False, debug=False)

    x = nc.dram_tensor("x", [bc, D], f32, kind="ExternalInput").ap()
    h = nc.dram_tensor("h", [bc, U], f32, kind="ExternalInput").ap()
    c_in = nc.dram_tensor("c", [bc, U], f32, kind="ExternalInput").ap()
    dp = nc.dram_tensor("dp_u", [bc, D], f32, kind="ExternalInput").ap()
    rdp = nc.dram_tensor("rec_dp_u", [bc, U], f32, kind="ExternalInput").ap()
    kw = nc.dram_tensor("kern", [D, NG4], f32, kind="ExternalInput").ap()
    rkw = nc.dram_tensor("rkern", [U, NG4], f32, kind="ExternalInput").ap()
    kdp = nc.dram_tensor("k_dp_u", [D, NG4], f32, kind="ExternalInput").ap()
    rkdp = nc.dram_tensor("rk_dp_u", [U, NG4], f32, kind="ExternalInput").ap()
    bias = nc.dram_tensor("bias", [NG4], f32, kind="ExternalInput").ap()
    h_new = nc.dram_tensor("h_new", [bc, U], f32, kind="ExternalOutput").ap()
    c_new = nc.dram_tensor("c_new", [bc, U], f32, kind="ExternalOutput").ap()

    with tile.TileContext(nc) as tc, ExitStack() as ctx:
        const = ctx.enter_context(tc.tile_pool(name="const", bufs=1))
        astage = ctx.enter_context(tc.tile_pool(name="astage", bufs=4))
        atrans = ctx.enter_context(tc.tile_pool(name="atrans", bufs=1))
        wstage = ctx.enter_context(tc.tile_pool(name="wstage", bufs=8))
        wmpool = ctx.enter_context(tc.tile_pool(name="wm", bufs=4))
        gstage = ctx.enter_context(tc.tile_pool(name="gstage", bufs=8))
        tpool = ctx.enter_context(tc.tile_pool(name="tpool", bufs=1))
        psum = ctx.enter_context(tc.tile_pool(name="psum", bufs=8, space="PSUM"))

        ident = const.tile([P, P], f32)
        make_identity(nc, ident)
        # f32r-typed operands: fp32r matmuls require producers that round to
        # fp32r, so every matmul operand tile is written with dtype float32r.
        ones_raw = const.tile([1, P], f32)
        nc.vector.memset(ones_raw, 1.0)
        ones1 = const.tile([1, P], f32r)
        nc.vector.tensor_copy(ones1, ones_raw)
        bias_raw = const.tile([1, NG4], f32)
        nc.sync.dma_start(out=bias_raw, in_=bias.unsqueeze(0))
        bias_s = const.tile([1, NG4], f32r)
        nc.vector.tensor_scalar_mul(bias_s, bias_raw, 1.0 / S)

        for _rep in range(repeat):
            emit_body(
                nc, tc, btl, bc,
                x, h, c_in, dp, rdp, kw, rkw, kdp, rkdp, h_new, c_new,
                astage, atrans, wstage, wmpool, gstage, tpool, psum,
                ident, ones1, bias_s,
            )

    nc.compile()
    return nc


def emit_body(
    nc, tc, btl, bc,
    x, h, c_in, dp, rdp, kw, rkw, kdp, rkdp, h_new, c_new,
    astage, atrans, wstage, wmpool, gstage, tpool, psum,
    ident, ones1, bias_s,
):
    # Per-(kk, bt) transposed-activation tiles [128, 128] so matmuls can begin
    # as soon as the slices they need exist (fine-grained deps).
    actT = [
        [
            atrans.tile([P, P], f32r, name=f"actT_{kk}_{bt}", tag=f"aT{kk}_{bt}")
            for bt in range(btl)
        ]
        for kk in range(KT)
    ]

    # ---- Phase 1: mask activations, transpose them into actT ----
    for bt in range(btl):
        rows = slice(bt * P, (bt + 1) * P)
        for src, usrc, kbase in ((x, dp, 0), (h, rdp, KT // 2)):
            vt = astage.tile([P, D], f32, tag="araw")
            ut = astage.tile([P, D], f32, tag="araw")
            nc.sync.dma_start(out=vt, in_=src[rows, :])
            nc.sync.dma_start(out=ut, in_=usrc[rows, :])
            vm = astage.tile([P, D], f32, tag="amask")
            nc.vector.scalar_tensor_tensor(
                vm, ut, DROPOUT, vt, op0=OP.is_ge, op1=OP.mult
            )
            for grp in range(2):
                pt = psum.tile([P, 4, P], f32, tag="ps", name=f"pt{bt}_{kbase}_{grp}")
                for q in range(4):
                    j = grp * 4 + q
                    nc.tensor.transpose(pt[:, q, :], vm[:, j * P : (j + 1) * P], ident)
                    nc.scalar.copy(actT[kbase + grp * 4 + q][bt], pt[:, q, :])

    # ---- Phase 2: matmul chunks + gate math, in gate order c~, i, f, o ----
    # Each 512-col chunk runs as two half-passes of 4 b-tiles, so each half's
    # gate math (DVE) overlaps the other half's k-loop and PSUM stays at 8
    # banks; weight-mask tiles are built during the first half and reused.
    # T_all[:, b, :] holds tanh(zc), then i*tanh(zc), then c', per b-tile.
    T_all = tpool.tile([P, btl, U], f32)
    bhalf = max(1, btl // 2)

    gate_order = [(2, 0), (2, 1), (0, 0), (0, 1), (1, 0), (1, 1), (3, 0), (3, 1)]
    for ci, (g, hf) in enumerate(gate_order):
        col0 = g * U + hf * NW
        ucols = slice(hf * NW, (hf + 1) * NW)
        wm = [None] * KT
        for bh in range(0, btl, bhalf):
            bs = range(bh, min(bh + bhalf, btl))
            zp = {
                b: psum.tile([P, NW], f32, tag="ps", name=f"z{ci}_{b}") for b in bs
            }
            for b in bs:
                nc.tensor.matmul(
                    zp[b],
                    lhsT=ones1,
                    rhs=bias_s[:, col0 : col0 + NW],
                    start=True,
                    stop=False,
                )
            for kk in range(KT):
                if bh == 0:
                    wsrc = kw if kk < KT // 2 else rkw
                    usrc = kdp if kk < KT // 2 else rkdp
                    r0 = (kk % (KT // 2)) * P
                    wt = wstage.tile([P, NW], f32, tag="wraw")
                    uw = wstage.tile([P, NW], f32, tag="wraw")
                    nc.sync.dma_start(out=wt, in_=wsrc[r0 : r0 + P, col0 : col0 + NW])
                    nc.sync.dma_start(out=uw, in_=usrc[r0 : r0 + P, col0 : col0 + NW])
                    wmt = wmpool.tile([P, NW], f32r, tag="wm", name=f"wm{ci}_{kk}")
                    nc.vector.scalar_tensor_tensor(
                        wmt, uw, KERNEL_DROPOUT, wt, op0=OP.is_ge, op1=OP.mult
                    )
                    wm[kk] = wmt
                for b in bs:
                    nc.tensor.matmul(
                        zp[b],
                        lhsT=actT[kk][b],
                        rhs=wm[kk],
                        start=False,
                        stop=(kk == KT - 1),
                    )
            for b in bs:
                rows = slice(b * P, (b + 1) * P)
                tsl = T_all[:, b, ucols]
                if g == 2:  # candidate: T = tanh(zc)
                    nc.scalar.activation(tsl, zp[b], AF.Tanh, scale=S)
                elif g == 0:  # input gate: T = sig(zi) * T
                    sg = gstage.tile([P, NW], f32, tag="g")
                    nc.scalar.activation(sg, zp[b], AF.Sigmoid, scale=S)
                    nc.vector.tensor_tensor(tsl, sg, tsl, OP.mult)
                elif g == 1:  # forget gate: c' = sig(zf)*c + T, store c'
                    sg = gstage.tile([P, NW], f32, tag="g")
                    nc.scalar.activation(sg, zp[b], AF.Sigmoid, scale=S)
                    ct = gstage.tile([P, NW], f32, tag="g")
                    nc.sync.dma_start(out=ct, in_=c_in[rows, ucols])
                    nc.vector.tensor_tensor(sg, sg, ct, OP.mult)
                    nc.vector.tensor_tensor(tsl, tsl, sg, OP.add)
                    nc.sync.dma_start(out=c_new[rows, ucols], in_=tsl)
                else:  # output gate: h' = sig(zo) * tanh(c')
                    sg = gstage.tile([P, NW], f32, tag="g")
                    nc.scalar.activation(sg, zp[b], AF.Sigmoid, scale=S)
                    tct = gstage.tile([P, NW], f32, tag="g")
                    nc.scalar.activation(tct, tsl, AF.Tanh)
                    nc.vector.tensor_tensor(tct, sg, tct, OP.mult)
                    nc.sync.dma_start(out=h_new[rows, ucols], in_=tct)


_NC_CACHE: dict[tuple, object] = {}


def get_nc(bc: int = BC, repeat: int = 1):
    key = (bc, repeat)
    if key not in _NC_CACHE:
        _NC_CACHE[key] = build_nc(bc, repeat)
    return _NC_CACHE[key]


def make_in_maps(x, h, c, kernel, recurrent_kernel, bias, dp_u, rec_dp_u, k_dp_u, rk_dp_u):
    def f(a):
        return np.ascontiguousarray(np.asarray(a, dtype=np.float32))

    kernel = f(kernel)
    recurrent_kernel = f(recurrent_kernel)
    bias = f(bias)
    k_dp_u = f(k_dp_u)
    rk_dp_u = f(rk_dp_u)
    x, h, c, dp_u, rec_dp_u = f(x), f(h), f(c), f(dp_u), f(rec_dp_u)

    in_maps = []
    for ci in range(N_CORES):
        sl = slice(ci * BC, (ci + 1) * BC)
        in_maps.append(
            {
                "x": np.ascontiguousarray(x[sl]),
                "h": np.ascontiguousarray(h[sl]),
                "c": np.ascontiguousarray(c[sl]),
                "dp_u": np.ascontiguousarray(dp_u[sl]),
                "rec_dp_u": np.ascontiguousarray(rec_dp_u[sl]),
                "kern": kernel,
                "rkern": recurrent_kernel,
                "k_dp_u": k_dp_u,
                "rk_dp_u": rk_dp_u,
                "bias": bias,
            }
        )
    return in_maps


def kernel(x, h, c, kernel, recurrent_kernel, bias, dp_u, rec_dp_u, k_dp_u, rk_dp_u):
    nc = get_nc()
    in_maps = make_in_maps(
        x, h, c, kernel, recurrent_kernel, bias, dp_u, rec_dp_u, k_dp_u, rk_dp_u
    )
    res = run_bass_kernel_spmd(nc, in_maps, core_ids=list(range(N_CORES)))
    h_new = np.concatenate([res.results[ci]["h_new"] for ci in range(N_CORES)], axis=0)
    c_new = np.concatenate([res.results[ci]["c_new"] for ci in range(N_CORES)], axis=0)
    return h_new, c_new
